# revision 7
# baseline (speedup 1.0000x reference)
"""CrossAttention kernel for 8x Trainium2 NeuronCores (Bass/Tile).

Reference computation (per batch b):
    q = rope(x @ Wq + bq)  [L, D] -> heads [H, L, HD]
    k = enc @ Wk + bk      [LE, D] -> [H, LE, HD]
    v = enc @ Wv + bv
    out = softmax(q k^T / sqrt(HD)) v  -> concat heads -> @ Wo + bo

Sharding: DP=4 over batch x TP=2 over head-groups. Core c handles batch
(c % 4) and heads [ (c//4)*8 , (c//4)*8+8 ). Each core produces a partial
[L, D] output (row-parallel Wo); host sums the two partials per batch and
adds bo.

Device-side layout choices (all matmuls bf16 inputs, fp32 PSUM accum):
  - host passes x^T and enc^T so the contraction dim is already on
    partitions; no on-device transposes needed anywhere.
  - scores are computed transposed (S^T[m, l]) so that P^T = exp(S^T) is
    directly the moving operand of the ctx^T matmul with V as stationary.
  - softmax skips max-subtraction: scores are ~N(0,1) bounded by ~6 for
    this problem's input distribution, exp is safe in fp32/bf16.
  - 1/sqrt(HD) and the rope pair-sign are baked into host-built cos/sin
    tables; rope pair-swap is a DVE stream_shuffle (mask swaps adjacent
    partitions within each 32-lane quadrant).
"""

import os

import numpy as np
import ml_dtypes

B, L, D = 4, 256, 2048
LE, DE = 2048, 1024
H = 16
HD = D // H  # 128
ROPE_BASE = 10000.0

P = 128
NCORES = 8
HN = H // 2          # heads per core (TP=2)
DC = HN * HD         # 1024 local head dims per core
KCQ = D // P         # 16 k-chunks for Q projection
KCE = DE // P        # 8 k-chunks for K/V projections
MC = LE // P         # 16 key chunks
MW = LE // 512       # 4 key windows for K^T projection
NW = D // 512        # 4 output column windows
LC = L // P          # 2 query-row chunks

BF16 = ml_dtypes.bfloat16

_CACHE = {}
LAST_RESULTS = None  # BassKernelResults of the most recent run (for test.py)


def _build_nc():
    import concourse.bass as bass  # noqa: F401
    import concourse.mybir as mybir
    import concourse.tile as tile
    from concourse import bacc

    f32 = mybir.dt.float32
    bf16 = mybir.dt.bfloat16
    AF = mybir.ActivationFunctionType
    OP = mybir.AluOpType

    nc = bacc.Bacc("TRN2", target_bir_lowering=False, debug=False)

    xT = nc.dram_tensor("xT", [D, L], bf16, kind="ExternalInput").ap()
    encT = nc.dram_tensor("encT", [DE, LE], bf16, kind="ExternalInput").ap()
    wq = nc.dram_tensor("wq", [D, DC], bf16, kind="ExternalInput").ap()
    wk = nc.dram_tensor("wk", [DE, DC], bf16, kind="ExternalInput").ap()
    wv = nc.dram_tensor("wv", [DE, DC], bf16, kind="ExternalInput").ap()
    wo = nc.dram_tensor("wo", [DC, D], bf16, kind="ExternalInput").ap()
    bq = nc.dram_tensor("bq", [DC], f32, kind="ExternalInput").ap()
    bk = nc.dram_tensor("bk", [DC], f32, kind="ExternalInput").ap()
    bv = nc.dram_tensor("bv", [1, DC], f32, kind="ExternalInput").ap()
    cost = nc.dram_tensor("cost", [P, L], f32, kind="ExternalInput").ap()
    sint = nc.dram_tensor("sint", [P, L], f32, kind="ExternalInput").ap()
    onescol = nc.dram_tensor("onescol", [P, 1], bf16, kind="ExternalInput").ap()
    onesrow = nc.dram_tensor("onesrow", [1, P], f32, kind="ExternalInput").ap()
    out = nc.dram_tensor("out", [L, D], f32, kind="ExternalOutput").ap()

    swap_mask = [i ^ 1 for i in range(32)]

    with tile.TileContext(nc) as tc:
        from contextlib import ExitStack

        with ExitStack() as ctx:
            const = ctx.enter_context(tc.tile_pool(name="const", bufs=1))
            keep = ctx.enter_context(tc.tile_pool(name="keep", bufs=1))
            work = ctx.enter_context(tc.tile_pool(name="work", bufs=2))
            ptpool = ctx.enter_context(tc.tile_pool(name="ptpool", bufs=6))
            ps_pp = ctx.enter_context(tc.tile_pool(name="ps_pp", bufs=2, space="PSUM"))
            ps_s = ctx.enter_context(tc.tile_pool(name="ps_s", bufs=2, space="PSUM"))
            ps_c = ctx.enter_context(tc.tile_pool(name="ps_c", bufs=2, space="PSUM"))
            ps_m = ctx.enter_context(tc.tile_pool(name="ps_m", bufs=1, space="PSUM"))

            # --- constants ---
            cos_sb = const.tile([P, L], f32, tag="cos")
            sin_sb = const.tile([P, L], f32, tag="sin")
            nc.sync.dma_start(cos_sb, cost)
            nc.sync.dma_start(sin_sb, sint)
            onesc_sb = const.tile([P, 1], bf16, tag="onesc")
            nc.sync.dma_start(onesc_sb, onescol)
            onesr_sb = const.tile([1, P], f32, tag="onesr")
            nc.sync.dma_start(onesr_sb, onesrow)
            bq_sb = const.tile([P, HN], f32, tag="bq")
            nc.sync.dma_start(bq_sb, bq.rearrange("(h p) -> p h", p=P))
            bk_sb = const.tile([P, HN], f32, tag="bk")
            nc.sync.dma_start(bk_sb, bk.rearrange("(h p) -> p h", p=P))
            bv_sb = const.tile([1, DC], f32, tag="bv")
            nc.sync.dma_start(bv_sb, bv)

            # broadcast bv along partitions via K=1 fp32 matmul
            bvbc_sb = const.tile([P, DC], f32, tag="bvbc")
            for nh in range(2):
                bvps = ps_m.tile([P, 512], f32, tag="sbc")
                nc.tensor.matmul(
                    bvps, lhsT=onesr_sb, rhs=bv_sb[:, nh * 512:(nh + 1) * 512],
                    start=True, stop=True,
                )
                nc.scalar.copy(bvbc_sb[:, nh * 512:(nh + 1) * 512], bvps)

            # --- persistent activation tensors ---
            kT_sb = keep.tile([P, HN, LE], bf16, tag="kT")      # K^T per head
            v_sb = keep.tile([P, MC, DC], bf16, tag="v")        # V  [m, d]
            qrot_sb = keep.tile([P, HN, L], bf16, tag="qrot")   # rope(Q)^T
            ctxn_sb = keep.tile([P, HN, L], bf16, tag="ctxn")   # normalized ctx^T

            with tc.tile_pool(name="phase1", bufs=1) as ph1:
                encT_sb = ph1.tile([P, KCE, LE], bf16, tag="encT")
                for kc in range(KCE):
                    nc.sync.dma_start(
                        encT_sb[:, kc, :], encT[kc * P:(kc + 1) * P, :]
                    )
                wk_sb = ph1.tile([P, KCE, DC], bf16, tag="wk")
                for kc in range(KCE):
                    nc.sync.dma_start(
                        wk_sb[:, kc, :], wk[kc * P:(kc + 1) * P, :]
                    )

                with tc.tile_pool(name="qin", bufs=1) as qin:
                    xT_sb = qin.tile([P, KCQ, L], bf16, tag="xT")
                    for kc in range(KCQ):
                        nc.sync.dma_start(
                            xT_sb[:, kc, :], xT[kc * P:(kc + 1) * P, :]
                        )
                    wq_sb = qin.tile([P, KCQ, DC], bf16, tag="wq")
                    for kc in range(KCQ):
                        nc.sync.dma_start(
                            wq_sb[:, kc, :], wq[kc * P:(kc + 1) * P, :]
                        )

                    # --- Q projection + rope ---
                    for h in range(HN):
                        qps = ps_pp.tile([P, 512], f32, tag="pp")
                        for kc in range(KCQ):
                            nc.tensor.matmul(
                                qps[:, :L],
                                lhsT=wq_sb[:, kc, h * P:(h + 1) * P],
                                rhs=xT_sb[:, kc, :],
                                start=(kc == 0),
                                stop=(kc == KCQ - 1),
                            )
                        qf = work.tile([P, L], f32, tag="qf")
                        nc.scalar.activation(
                            qf, qps[:, :L], AF.Identity, bias=bq_sb[:, h:h + 1]
                        )
                        qs = work.tile([P, L], f32, tag="qs")
                        nc.vector.stream_shuffle(qs, qf, swap_mask)
                        t1 = work.tile([P, L], f32, tag="t1")
                        nc.vector.tensor_tensor(t1, qf, cos_sb, op=OP.mult)
                        t2 = work.tile([P, L], f32, tag="t2")
                        nc.vector.tensor_tensor(t2, qs, sin_sb, op=OP.mult)
                        nc.vector.tensor_tensor(qrot_sb[:, h, :], t1, t2, op=OP.add)

                with tc.tile_pool(name="vin", bufs=1) as vin:
                    wv_sb = vin.tile([P, KCE, DC], bf16, tag="wv")
                    for kc in range(KCE):
                        nc.sync.dma_start(
                            wv_sb[:, kc, :], wv[kc * P:(kc + 1) * P, :]
                        )

                    # --- K^T projection ---
                    for h in range(HN):
                        for w in range(MW):
                            kps = ps_pp.tile([P, 512], f32, tag="pp")
                            for kc in range(KCE):
                                nc.tensor.matmul(
                                    kps,
                                    lhsT=wk_sb[:, kc, h * P:(h + 1) * P],
                                    rhs=encT_sb[:, kc, w * 512:(w + 1) * 512],
                                    start=(kc == 0),
                                    stop=(kc == KCE - 1),
                                )
                            nc.scalar.activation(
                                kT_sb[:, h, w * 512:(w + 1) * 512],
                                kps,
                                AF.Identity,
                                bias=bk_sb[:, h:h + 1],
                            )

                    # --- V projection ---
                    for mc in range(MC):
                        for nh in range(2):
                            vps = ps_pp.tile([P, 512], f32, tag="pp")
                            for kc in range(KCE):
                                nc.tensor.matmul(
                                    vps,
                                    lhsT=encT_sb[:, kc, mc * P:(mc + 1) * P],
                                    rhs=wv_sb[:, kc, nh * 512:(nh + 1) * 512],
                                    start=(kc == 0),
                                    stop=(kc == KCE - 1),
                                )
                            nc.vector.tensor_tensor(
                                v_sb[:, mc, nh * 512:(nh + 1) * 512],
                                vps,
                                bvbc_sb[:, nh * 512:(nh + 1) * 512],
                                op=OP.add,
                            )

            # phase1 inputs are dead; wo reuses the space
            with tc.tile_pool(name="phase2", bufs=1) as ph2:
                wo_sb = ph2.tile([P, HN, D], bf16, tag="wo")
                for h in range(HN):
                    nc.sync.dma_start(
                        wo_sb[:, h, :], wo[h * P:(h + 1) * P, :]
                    )

                # --- attention per head ---
                for h in range(HN):
                    ctxps = ps_c.tile([P, L], f32, tag="ctx")
                    sums = ps_m.tile([1, L], f32, tag="sums")
                    for mc in range(MC):
                        sps = ps_s.tile([P, L], f32, tag="sps")
                        nc.tensor.matmul(
                            sps,
                            lhsT=kT_sb[:, h, mc * P:(mc + 1) * P],
                            rhs=qrot_sb[:, h, :],
                            start=True,
                            stop=True,
                        )
                        pt = ptpool.tile([P, L], bf16, tag="pt")
                        nc.scalar.activation(pt, sps, AF.Exp)
                        nc.tensor.matmul(
                            ctxps,
                            lhsT=v_sb[:, mc, h * P:(h + 1) * P],
                            rhs=pt,
                            start=(mc == 0),
                            stop=(mc == MC - 1),
                        )
                        nc.tensor.matmul(
                            sums,
                            lhsT=onesc_sb,
                            rhs=pt,
                            start=(mc == 0),
                            stop=(mc == MC - 1),
                        )
                    recip = work.tile([1, L], f32, tag="recip")
                    nc.vector.reciprocal(recip, sums)
                    sbcps = ps_m.tile([P, L], f32, tag="sbc")
                    nc.tensor.matmul(
                        sbcps, lhsT=onesr_sb, rhs=recip, start=True, stop=True
                    )
                    sbc = work.tile([P, L], f32, tag="sbc_sb")
                    nc.scalar.copy(sbc, sbcps)
                    nc.vector.tensor_tensor(
                        ctxn_sb[:, h, :], ctxps, sbc, op=OP.mult
                    )

                # --- output projection (partial over local heads) ---
                for lc in range(LC):
                    for nw in range(NW):
                        ops = ps_pp.tile([P, 512], f32, tag="pp")
                        for h in range(HN):
                            nc.tensor.matmul(
                                ops,
                                lhsT=ctxn_sb[:, h, lc * P:(lc + 1) * P],
                                rhs=wo_sb[:, h, nw * 512:(nw + 1) * 512],
                                start=(h == 0),
                                stop=(h == HN - 1),
                            )
                        osb = work.tile([P, 512], f32, tag="osb")
                        nc.scalar.copy(osb, ops)
                        nc.sync.dma_start(
                            out[lc * P:(lc + 1) * P, nw * 512:(nw + 1) * 512],
                            osb,
                        )

    nc.compile()
    return nc


def _rope_tables():
    half = HD // 2
    inv_freq = 1.0 / (ROPE_BASE ** (np.arange(0, HD, 2, dtype=np.float64) / HD))
    pos = np.arange(L, dtype=np.float64)
    ang = pos[None, :] * inv_freq[:, None]  # [half, L]
    sc = 1.0 / np.sqrt(np.float64(HD))
    cos_t = np.empty((P, L), dtype=np.float32)
    sin_t = np.empty((P, L), dtype=np.float32)
    c = (np.cos(ang) * sc).astype(np.float32)
    s = (np.sin(ang) * sc).astype(np.float32)
    cos_t[0::2, :] = c
    cos_t[1::2, :] = c
    sin_t[0::2, :] = -s
    sin_t[1::2, :] = s
    return cos_t, sin_t


def prepare_in_maps(x, enc, Wq, bq, Wk, bk, Wv, bv, Wo):
    cos_t, sin_t = _rope_tables()
    onescol = np.ones((P, 1), dtype=BF16)
    onesrow = np.ones((1, P), dtype=np.float32)

    in_maps = []
    for c in range(NCORES):
        b = c % B
        g = c // B
        sl = slice(g * DC, (g + 1) * DC)
        in_maps.append({
            "xT": np.ascontiguousarray(x[b].T).astype(BF16),
            "encT": np.ascontiguousarray(enc[b].T).astype(BF16),
            "wq": np.ascontiguousarray(Wq[:, sl]).astype(BF16),
            "wk": np.ascontiguousarray(Wk[:, sl]).astype(BF16),
            "wv": np.ascontiguousarray(Wv[:, sl]).astype(BF16),
            "wo": np.ascontiguousarray(Wo[sl, :]).astype(BF16),
            "bq": np.ascontiguousarray(bq[sl]),
            "bk": np.ascontiguousarray(bk[sl]),
            "bv": np.ascontiguousarray(bv[sl])[None, :],
            "cost": cos_t,
            "sint": sin_t,
            "onescol": onescol,
            "onesrow": onesrow,
        })
    return in_maps


def kernel(x, encoder_inputs, Wq, bq, Wk, bk, Wv, bv, Wo, bo):
    global LAST_RESULTS
    from concourse.bass_utils import run_bass_kernel_spmd

    x = np.asarray(x, dtype=np.float32)
    enc = np.asarray(encoder_inputs, dtype=np.float32)
    Wq = np.asarray(Wq, dtype=np.float32)
    Wk = np.asarray(Wk, dtype=np.float32)
    Wv = np.asarray(Wv, dtype=np.float32)
    Wo = np.asarray(Wo, dtype=np.float32)
    bq = np.asarray(bq, dtype=np.float32)
    bk = np.asarray(bk, dtype=np.float32)
    bv = np.asarray(bv, dtype=np.float32)
    bo = np.asarray(bo, dtype=np.float32)

    if "nc" not in _CACHE:
        _CACHE["nc"] = _build_nc()
    nc = _CACHE["nc"]

    in_maps = prepare_in_maps(x, enc, Wq, bq, Wk, bk, Wv, bv, Wo)

    trace = bool(int(os.environ.get("KERNEL_TRACE", "0")))
    try:
        res = run_bass_kernel_spmd(
            nc, in_maps, core_ids=list(range(NCORES)), trace=trace
        )
    except ModuleNotFoundError:
        # NTFF profiling hook unavailable (axon client without antenv hooks)
        res = run_bass_kernel_spmd(
            nc, in_maps, core_ids=list(range(NCORES)), trace=False
        )
    LAST_RESULTS = res

    out = np.empty((B, L, D), dtype=np.float32)
    for b in range(B):
        out[b] = res.results[b]["out"] + res.results[b + B]["out"] + bo[None, :]
    return out
